# revision 5
# baseline (speedup 1.0000x reference)
"""Trainium2 Bass kernel for the additive-attention layer.

reference math (B=32, L=4096, E=512, H=512, A=256):
  enc  = features @ W_enc + b_enc                  [B,L,A]
  dec  = decoder_hidden @ W_dec + b_dec            [B,A]
  sim  = relu(enc + dec[:,None,:])                 [B,L,A]
  s    = (sim @ W_v + b_v)[...,0]                  [B,L]   (b_v drops: softmax shift-invariant)
  w    = softmax(s, axis=1)                        [B,L]
  out  = einsum('ble,bl->be', features, w)         [B,E]
  returns (out, w)

Strategy: data-parallel over batch, 4 batches per NeuronCore, no collectives.
Host ships features transposed ([Bloc, E, L]) in bf16 so the contraction dim
sits on SBUF partitions (no on-device transposes) and DMA traffic is halved.
"""

import os
import numpy as np
from contextlib import ExitStack

B, L, E, H, A = 32, 4096, 512, 512, 256
NCORES = 8
BLOC = B // NCORES  # 4

KCH = E // 128   # 4 contraction blocks for enc
MCH = A // 128   # 2 output-partition chunks for enc / contraction blocks for scores
NCH = 8          # l-chunks of 512
NSZ = L // NCH   # 512

_CACHE = {}
LAST_RESULT = None


def _build(pass2_mode="stt"):
    import concourse.bass as bass  # noqa: F401
    from concourse import bacc, mybir
    import concourse.tile as tile
    from concourse.masks import make_identity

    BF = mybir.dt.bfloat16
    F32 = mybir.dt.float32
    AF = mybir.ActivationFunctionType
    OP = mybir.AluOpType

    nc = bacc.Bacc("TRN2", target_bir_lowering=False, debug=False, num_devices=NCORES)

    fT = nc.dram_tensor("fT", [BLOC, E, L], BF, kind="ExternalInput").ap()
    dhT = nc.dram_tensor("dhT", [H, BLOC], BF, kind="ExternalInput").ap()
    w_enc = nc.dram_tensor("w_enc", [E, A], BF, kind="ExternalInput").ap()
    w_dec = nc.dram_tensor("w_dec", [H, A], BF, kind="ExternalInput").ap()
    wv_rep = nc.dram_tensor("wv_rep", [A, 128], BF, kind="ExternalInput").ap()
    b_enc_d = nc.dram_tensor("b_enc", [1, A], F32, kind="ExternalInput").ap()
    b_dec_d = nc.dram_tensor("b_dec", [1, A], F32, kind="ExternalInput").ap()
    out_d = nc.dram_tensor("out", [BLOC * 4, 128], F32, kind="ExternalOutput").ap()
    attn_d = nc.dram_tensor("attn", [BLOC, L], F32, kind="ExternalOutput").ap()

    with TileBlock(nc, tile) as (tc, ctx):
        consts = ctx.enter_context(tc.tile_pool(name="consts", bufs=1))
        fpool = ctx.enter_context(tc.tile_pool(name="fpool", bufs=8))
        sim_pool = ctx.enter_context(tc.tile_pool(name="simp", bufs=6))
        attn_pool = ctx.enter_context(tc.tile_pool(name="attnp", bufs=2))
        misc = ctx.enter_context(tc.tile_pool(name="misc", bufs=2))
        keep = ctx.enter_context(tc.tile_pool(name="keep", bufs=1))
        penc = ctx.enter_context(tc.tile_pool(name="penc", bufs=4, space="PSUM"))
        psc = ctx.enter_context(tc.tile_pool(name="psc", bufs=2, space="PSUM"))
        pmisc = ctx.enter_context(tc.tile_pool(name="pmisc", bufs=1, space="PSUM"))

        # ---- constants / weights ----
        wenc_sb = []
        for k in range(KCH):
            t = consts.tile([128, A], BF, tag=f"wenc{k}")
            nc.sync.dma_start(out=t[:], in_=w_enc[k * 128:(k + 1) * 128, :])
            wenc_sb.append(t)
        wdec_sb = []
        for k in range(KCH):
            t = consts.tile([128, A], BF, tag=f"wdec{k}")
            nc.sync.dma_start(out=t[:], in_=w_dec[k * 128:(k + 1) * 128, :])
            wdec_sb.append(t)
        wv_sb = []
        for m in range(MCH):
            t = consts.tile([128, 128], BF, tag=f"wv{m}")
            nc.sync.dma_start(out=t[:], in_=wv_rep[m * 128:(m + 1) * 128, :])
            wv_sb.append(t)
        dh_sb = []
        for k in range(KCH):
            t = consts.tile([128, BLOC], BF, tag=f"dh{k}")
            nc.sync.dma_start(out=t[:], in_=dhT[k * 128:(k + 1) * 128, :])
            dh_sb.append(t)
        be_sb = consts.tile([1, A], F32, tag="be")
        nc.sync.dma_start(out=be_sb[:], in_=b_enc_d[:])
        bd_sb = consts.tile([1, A], F32, tag="bd")
        nc.sync.dma_start(out=bd_sb[:], in_=b_dec_d[:])
        bsum = consts.tile([1, A], BF, tag="bsum")
        nc.vector.tensor_add(bsum[:], be_sb[:], bd_sb[:])
        ones4 = consts.tile([1, BLOC], BF, tag="ones4")
        nc.vector.memset(ones4[:], 1.0)
        ident = consts.tile([128, 128], F32, tag="ident")
        make_identity(nc, ident[:])

        # ---- decoder bias columns: cT[m][a,b] = (dh @ W_dec + b_enc + b_dec)^T ----
        cT_sb = []
        for m in range(MCH):
            p = pmisc.tile([128, BLOC], F32, tag="pcT")
            for k in range(KCH):
                nc.tensor.matmul(
                    p[:], lhsT=wdec_sb[k][:, m * 128:(m + 1) * 128], rhs=dh_sb[k][:],
                    start=(k == 0), stop=False,
                )
            nc.tensor.matmul(
                p[:], lhsT=bsum[:, m * 128:(m + 1) * 128], rhs=ones4[:],
                start=False, stop=True,
            )
            t = consts.tile([128, BLOC], F32, tag=f"cT{m}")
            nc.scalar.copy(t[:], p[:])
            cT_sb.append(t)

        # ---- main loop over local batches ----
        out_cols = keep.tile([128, BLOC * 4], F32, tag="outcols")
        scratch = keep.tile([128, L], BF, tag="ttr_scratch")

        for b in range(BLOC):
            fsb = []
            for e in range(KCH):
                t = fpool.tile([128, L], BF, tag="F")
                nc.sync.dma_start(out=t[:], in_=fT[b, e * 128:(e + 1) * 128, :])
                fsb.append(t)

            attn_b = attn_pool.tile([128, L], BF, tag="attn_b")
            zp = misc.tile([128, NCH], F32, tag="zp")

            prev = None  # (sims, nch) pending scores computation

            def emit_scores(sims, nch):
                ps = psc.tile([128, NSZ], F32, tag="psc")
                for m in range(MCH):
                    nc.tensor.matmul(
                        ps[:], lhsT=wv_sb[m][:], rhs=sims[m][:],
                        start=(m == 0), stop=(m == MCH - 1),
                    )
                nc.scalar.activation(
                    attn_b[:, nch * NSZ:(nch + 1) * NSZ], ps[:], AF.Exp,
                    accum_out=zp[:, nch:nch + 1],
                )

            for nch in range(NCH):
                sims = []
                for m in range(MCH):
                    pe_ = penc.tile([128, NSZ], F32, tag="penc")
                    for k in range(KCH):
                        nc.tensor.matmul(
                            pe_[:],
                            lhsT=wenc_sb[k][:, m * 128:(m + 1) * 128],
                            rhs=fsb[k][:, nch * NSZ:(nch + 1) * NSZ],
                            start=(k == 0), stop=(k == KCH - 1),
                        )
                    sim = sim_pool.tile([128, NSZ], BF, tag="sim")
                    bias_ap = cT_sb[m][:, b:b + 1]
                    idx = nch * MCH + m
                    if idx % 3 == 1:  # ~1/3 of evacuations go to DVE
                        nc.vector.tensor_scalar(
                            sim[:], pe_[:], bias_ap, 0.0, op0=OP.add, op1=OP.max,
                        )
                    else:
                        nc.scalar.activation(sim[:], pe_[:], AF.Relu, bias=bias_ap)
                    sims.append(sim)
                if prev is not None:
                    emit_scores(*prev)
                prev = (sims, nch)
            emit_scores(*prev)

            z = misc.tile([128, 1], F32, tag="z")
            nc.vector.tensor_reduce(
                z[:], zp[:], axis=_ax(tile), op=OP.add,
            )
            rz = misc.tile([128, 1], F32, tag="rz")
            nc.vector.reciprocal(rz[:], z[:])

            # pass 2: out[e] = sum_l fT[e,l] * exp(s[l]);  normalized at the end
            p4 = misc.tile([128, KCH], F32, tag="p4")
            for e in range(KCH):
                if pass2_mode == "stt":
                    # fused: scratch = (F * 1.0) * attn_b, accum = row-sum
                    nc.vector.scalar_tensor_tensor(
                        out=scratch[:], in0=fsb[e][:], scalar=1.0,
                        in1=attn_b[:], op0=OP.mult, op1=OP.mult,
                        accum_out=p4[:, e:e + 1],
                    )
                elif pass2_mode == "mul_reduce":
                    nc.vector.tensor_mul(scratch[:], fsb[e][:], attn_b[:])
                    nc.vector.tensor_reduce(
                        p4[:, e:e + 1], scratch[:], axis=mybir.AxisListType.X,
                        op=OP.add,
                    )
                elif pass2_mode == "gp_mul_reduce":
                    nc.gpsimd.tensor_mul(scratch[:], fsb[e][:], attn_b[:])
                    nc.vector.tensor_reduce(
                        p4[:, e:e + 1], scratch[:], axis=mybir.AxisListType.X,
                        op=OP.add,
                    )
                else:
                    raise ValueError(pass2_mode)
            nc.vector.tensor_scalar_mul(
                out_cols[:, b * KCH:(b + 1) * KCH], p4[:], rz[:],
            )

            attn_n = attn_pool.tile([128, L], BF, tag="attn_n")
            nc.vector.tensor_scalar_mul(attn_n[:], attn_b[:], rz[:])
            nc.gpsimd.dma_start(out=attn_d[b:b + 1, :], in_=attn_n[0:1, :])

        # ---- epilogue: transpose out_cols -> [16, 128] and store ----
        pout = pmisc.tile([BLOC * 4, 128], F32, tag="pout")
        nc.tensor.transpose(pout[:], out_cols[:], ident[:])
        out_sb = keep.tile([BLOC * 4, 128], F32, tag="outsb")
        nc.scalar.copy(out_sb[:], pout[:])
        nc.sync.dma_start(out=out_d[:], in_=out_sb[:])

    nc.compile()
    return nc


def _ax(tile_mod):
    from concourse import mybir
    return mybir.AxisListType.X


class TileBlock:
    """Small helper so _build reads linearly: enters TileContext + ExitStack."""

    def __init__(self, nc, tile_mod):
        self.nc = nc
        self.tile_mod = tile_mod

    def __enter__(self):
        self._stack = ExitStack()
        tc = self._stack.enter_context(self.tile_mod.TileContext(self.nc))
        return tc, self._stack

    def __exit__(self, *exc):
        return self._stack.__exit__(*exc)


def kernel(features, decoder_hidden, W_enc, b_enc, W_dec, b_dec, W_v, b_v):
    global LAST_RESULT
    import ml_dtypes
    from concourse.bass_utils import run_bass_kernel_spmd

    mode = os.environ.get("KERNEL_PASS2_MODE", "stt")
    if mode not in _CACHE:
        _CACHE[mode] = _build(mode)
    nc = _CACHE[mode]

    bf16 = ml_dtypes.bfloat16
    features = np.asarray(features, dtype=np.float32)
    decoder_hidden = np.asarray(decoder_hidden, dtype=np.float32)
    W_enc = np.asarray(W_enc, dtype=np.float32)
    b_enc = np.asarray(b_enc, dtype=np.float32)
    W_dec = np.asarray(W_dec, dtype=np.float32)
    b_dec = np.asarray(b_dec, dtype=np.float32)
    W_v = np.asarray(W_v, dtype=np.float32)

    # one big transpose+cast, then per-core contiguous slices
    fT_all = np.ascontiguousarray(features.transpose(0, 2, 1)).astype(bf16)  # [B,E,L]
    wenc_b = W_enc.astype(bf16)
    wdec_b = W_dec.astype(bf16)
    wv_rep = np.repeat(W_v.astype(bf16), 128, axis=1)  # [A,128]
    be_r = b_enc.reshape(1, A)
    bd_r = b_dec.reshape(1, A)

    in_maps = []
    for i in range(NCORES):
        sl = slice(i * BLOC, (i + 1) * BLOC)
        in_maps.append({
            "fT": fT_all[sl],
            "dhT": np.ascontiguousarray(decoder_hidden[sl].T).astype(bf16),
            "w_enc": wenc_b,
            "w_dec": wdec_b,
            "wv_rep": wv_rep,
            "b_enc": be_r,
            "b_dec": bd_r,
        })

    res = run_bass_kernel_spmd(nc, in_maps, core_ids=list(range(NCORES)))
    LAST_RESULT = res

    out = np.empty((B, E), np.float32)
    attn = np.empty((B, L), np.float32)
    for i, r in enumerate(res.results):
        sl = slice(i * BLOC, (i + 1) * BLOC)
        out[sl] = r["out"].reshape(BLOC, E)
        attn[sl] = r["attn"]
    return out, attn
